# revision 9
# baseline (speedup 1.0000x reference)
"""Category-specific linear layer (MoE-style routing) on 8 Trainium2 cores.

Reference computation:
    out[s] = x[s] @ W[cat_ids[s]] + b[cat_ids[s]]
    x: [64, 256, 1024] f32, cat_ids: [64] int64,
    W: [16, 1024, 4096] f32, b: [16, 4096] f32  ->  out: [64, 256, 4096] f32

Strategy (data-parallel over batch, routing resolved on host):
  - cat_ids is a host-visible input, so the per-sample weight gather is done
    on the host: core c gets samples [8c, 8c+8) plus the 8 matching weight
    matrices, cast to fp16 (PE runs 16-bit matmuls at full rate; fp32 PSUM
    accumulation keeps the error ~1e-3).
  - x is pre-transposed on the host to [K, tokens] so it can serve as the
    stationary matmul operand without an on-chip transpose.
  - One uniform SPMD program for all 8 cores; per-core differences live
    entirely in the input data.
"""

import os
from contextlib import ExitStack

import numpy as np

NCORES = 8
B, T, I, H, C = 64, 256, 1024, 4096, 16
SPC = B // NCORES      # samples per core
TOK = SPC * T          # token rows per core
KT = I // 128          # contraction tiles
NFREE = 512            # matmul moving free dim (one PSUM bank of fp32)
NT = H // NFREE        # n tiles
MT_PER_S = T // 128    # m tiles per sample

_CACHE = {}


def _build_nc(spc=SPC, kt=KT, h=H, nt=NT, mt_per_s=MT_PER_S, nfree=NFREE):
    import concourse.tile as tile
    from concourse import bacc, mybir

    tok = spc * mt_per_s * 128
    nc = bacc.Bacc()
    xT = nc.dram_tensor("xT", [kt, 128, tok], mybir.dt.float16, kind="ExternalInput")
    Wg = nc.dram_tensor("Wg", [spc, kt, 128, h], mybir.dt.float16, kind="ExternalInput")
    out = nc.dram_tensor("out", [tok, h], mybir.dt.float16, kind="ExternalOutput")

    with ExitStack() as ctx:
        tc = ctx.enter_context(tile.TileContext(nc))
        xpool = ctx.enter_context(tc.tile_pool(name="xp", bufs=kt))
        wpool = ctx.enter_context(tc.tile_pool(name="wp", bufs=2 * kt))
        opool = ctx.enter_context(tc.tile_pool(name="op", bufs=2))
        pspool = ctx.enter_context(tc.tile_pool(name="ps", bufs=8, space="PSUM"))

        xtiles = []
        for k in range(kt):
            t = xpool.tile([128, tok], mybir.dt.float16, tag="xt")
            nc.sync.dma_start(t[:], xT[k, :, :])
            xtiles.append(t)

        for s in range(spc):
            wtiles = []
            for k in range(kt):
                wt = wpool.tile([128, h], mybir.dt.float16, tag="wt")
                nc.sync.dma_start(wt[:], Wg[s, k, :, :])
                wtiles.append(wt)
            for mi in range(mt_per_s):
                m = s * mt_per_s + mi
                ot = opool.tile([128, nt * nfree], mybir.dt.float16, tag="ot")
                # n tiles in two groups of nt//2 so PSUM eviction of one
                # group overlaps matmuls of the next (8 banks total).
                gsz = max(nt // 2, 1)
                for g in range(0, nt, gsz):
                    pts = [
                        pspool.tile([128, nfree], mybir.dt.float32, tag="ps",
                                    name=f"ps_{m}_{g}_{j}")
                        for j in range(gsz)
                    ]
                    for k in range(kt):
                        lhsT = xtiles[k][:, m * 128:(m + 1) * 128]
                        for j in range(gsz):
                            n = g + j
                            nc.tensor.matmul(
                                pts[j][:],
                                lhsT,
                                wtiles[k][:, n * nfree:(n + 1) * nfree],
                                start=(k == 0),
                                stop=(k == kt - 1),
                            )
                    for j in range(gsz):
                        n = g + j
                        nc.vector.tensor_copy(
                            ot[:, n * nfree:(n + 1) * nfree], pts[j][:]
                        )
                nc.gpsimd.dma_start(out[m * 128:(m + 1) * 128, :], ot[:])
    nc.compile()
    return nc


def _get_nc():
    if "nc" not in _CACHE:
        _CACHE["nc"] = _build_nc()
    return _CACHE["nc"]


def kernel(x, cat_ids, W, b):
    from concourse.bass_utils import run_bass_kernel_spmd

    x = np.asarray(x)
    cat_ids = np.asarray(cat_ids)
    W = np.asarray(W)
    b = np.asarray(b)

    W16 = W.astype(np.float16)
    in_maps = []
    for c in range(NCORES):
        sl = slice(c * SPC, (c + 1) * SPC)
        xs = x[sl].reshape(TOK, I).astype(np.float16)
        xT = np.ascontiguousarray(xs.T).reshape(KT, 128, TOK)
        Wg = np.ascontiguousarray(W16[cat_ids[sl]]).reshape(SPC, KT, 128, H)
        in_maps.append({"xT": xT, "Wg": Wg})

    nc = _get_nc()
    res = run_bass_kernel_spmd(nc, in_maps, core_ids=list(range(NCORES)))
    _CACHE["last_res"] = res

    out = np.empty((B, T, H), dtype=np.float32)
    for c in range(NCORES):
        out[c * SPC:(c + 1) * SPC] = (
            res.results[c]["out"].astype(np.float32).reshape(SPC, T, H)
        )
    if b.any():
        out += b[cat_ids].astype(np.float32)[:, None, :]
    return out


# revision 11
# speedup vs baseline: 1.0739x; 1.0739x over previous
"""Category-specific linear layer (MoE-style routing) on 8 Trainium2 cores.

Reference computation:
    out[s] = x[s] @ W[cat_ids[s]] + b[cat_ids[s]]
    x: [64, 256, 1024] f32, cat_ids: [64] int64,
    W: [16, 1024, 4096] f32, b: [16, 4096] f32  ->  out: [64, 256, 4096] f32

Strategy (data-parallel over batch, routing resolved on host):
  - cat_ids is a host-visible input, so the per-sample weight gather is done
    on the host: core c gets samples [8c, 8c+8) plus the 8 matching weight
    matrices, cast to fp16 (PE runs 16-bit matmuls at full rate; fp32 PSUM
    accumulation keeps the error ~1e-3).
  - x is pre-transposed on the host to [K, tokens] so it can serve as the
    stationary matmul operand without an on-chip transpose.
  - One uniform SPMD program for all 8 cores; per-core differences live
    entirely in the input data.
"""

import os
from contextlib import ExitStack

import numpy as np

NCORES = 8
B, T, I, H, C = 64, 256, 1024, 4096, 16
SPC = B // NCORES      # samples per core
TOK = SPC * T          # token rows per core
KT = I // 128          # contraction tiles
NFREE = 512            # matmul moving free dim (one PSUM bank of fp32)
NT = H // NFREE        # n tiles
MT_PER_S = T // 128    # m tiles per sample

_CACHE = {}


def _build_nc(spc=SPC, kt=KT, h=H, nt=NT, mt_per_s=MT_PER_S, nfree=NFREE):
    import concourse.tile as tile
    from concourse import bacc, mybir

    tok = spc * mt_per_s * 128
    nc = bacc.Bacc()
    xT = nc.dram_tensor("xT", [kt, 128, tok], mybir.dt.float16, kind="ExternalInput")
    Wg = nc.dram_tensor("Wg", [spc, kt, 128, h], mybir.dt.float16, kind="ExternalInput")
    out = nc.dram_tensor("out", [tok, h], mybir.dt.float16, kind="ExternalOutput")

    with ExitStack() as ctx:
        tc = ctx.enter_context(tile.TileContext(nc))
        xpool = ctx.enter_context(tc.tile_pool(name="xp", bufs=kt))
        wpool = ctx.enter_context(tc.tile_pool(name="wp", bufs=2 * kt + 2))
        opool = ctx.enter_context(tc.tile_pool(name="op", bufs=2))
        pspool = ctx.enter_context(tc.tile_pool(name="ps", bufs=8, space="PSUM"))

        # Stripe input DMAs across both HWDGE engines (sync + scalar): a
        # single HW queue sustains only ~285 GB/s, below what the weight
        # stream needs to stay ahead of the PE.
        dma_engines = [nc.sync, nc.scalar]

        xtiles = []
        for k in range(kt):
            t = xpool.tile([128, tok], mybir.dt.float16, tag="xt")
            dma_engines[k % 2].dma_start(t[:], xT[k, :, :])
            xtiles.append(t)

        for s in range(spc):
            wtiles = []
            for k in range(kt):
                wt = wpool.tile([128, h], mybir.dt.float16, tag="wt")
                dma_engines[k % 2].dma_start(wt[:], Wg[s, k, :, :])
                wtiles.append(wt)
            for mi in range(mt_per_s):
                m = s * mt_per_s + mi
                ot = opool.tile([128, nt * nfree], mybir.dt.float16, tag="ot")
                # n tiles in two groups of nt//2 so PSUM eviction of one
                # group overlaps matmuls of the next (8 banks total).
                gsz = max(nt // 2, 1)
                for g in range(0, nt, gsz):
                    pts = [
                        pspool.tile([128, nfree], mybir.dt.float32, tag="ps",
                                    name=f"ps_{m}_{g}_{j}")
                        for j in range(gsz)
                    ]
                    for k in range(kt):
                        lhsT = xtiles[k][:, m * 128:(m + 1) * 128]
                        for j in range(gsz):
                            n = g + j
                            nc.tensor.matmul(
                                pts[j][:],
                                lhsT,
                                wtiles[k][:, n * nfree:(n + 1) * nfree],
                                start=(k == 0),
                                stop=(k == kt - 1),
                            )
                    for j in range(gsz):
                        n = g + j
                        nc.vector.tensor_copy(
                            ot[:, n * nfree:(n + 1) * nfree], pts[j][:]
                        )
                nc.gpsimd.dma_start(out[m * 128:(m + 1) * 128, :], ot[:])
    nc.compile()
    return nc


def _get_nc():
    if "nc" not in _CACHE:
        _CACHE["nc"] = _build_nc()
    return _CACHE["nc"]


def kernel(x, cat_ids, W, b):
    from concourse.bass_utils import run_bass_kernel_spmd

    x = np.asarray(x)
    cat_ids = np.asarray(cat_ids)
    W = np.asarray(W)
    b = np.asarray(b)

    W16 = W.astype(np.float16)
    in_maps = []
    for c in range(NCORES):
        sl = slice(c * SPC, (c + 1) * SPC)
        xs = x[sl].reshape(TOK, I).astype(np.float16)
        xT = np.ascontiguousarray(xs.T).reshape(KT, 128, TOK)
        Wg = np.ascontiguousarray(W16[cat_ids[sl]]).reshape(SPC, KT, 128, H)
        in_maps.append({"xT": xT, "Wg": Wg})

    nc = _get_nc()
    res = run_bass_kernel_spmd(nc, in_maps, core_ids=list(range(NCORES)))
    _CACHE["last_res"] = res

    out = np.empty((B, T, H), dtype=np.float32)
    for c in range(NCORES):
        out[c * SPC:(c + 1) * SPC] = (
            res.results[c]["out"].astype(np.float32).reshape(SPC, T, H)
        )
    if b.any():
        out += b[cat_ids].astype(np.float32)[:, None, :]
    return out
